# revision 1
# baseline (speedup 1.0000x reference)
"""Trainium2 Bass kernel for nn_NeuralMemory (B=4, N=1024, D=128, DEPTH=4).

Sharding: 8 cores, core c handles batch b = c//2. The store phase
(per-token grads of the 4 memory weights, summed over the sequence) is
computed redundantly by both cores of a pair -- the grad sum is
permutation invariant over tokens, so each core is fed its batch's
sequence with its own retrieval half rotated to the front and retrieves
tokens [0:512) of its view. No cross-core communication (a pair-wise
AllReduce has a ~10us floor, worse than the duplicated compute).

Layout: activations are feature-major [D=128 partitions, tokens]; the
store phase runs in two 512-token tiles. Layer matmuls are
matmul(out^T, lhsT=W, rhs=X^T) with float32r operands (~2e-4
per-matmul rel err on HW). dW_i = A_i^T @ G_i contracts over tokens,
so A/G get bf16 copies rotated token-major via PE transposes (4 chunks
per PSUM bank); dW matmuls run bf16 with fp32 PSUM accumulation.
dW3/dW2/dW1 and M = S^T @ G0 share one PSUM bank (a single
accumulation group). Tile-1's H tiles borrow the dW-transpose ("tr")
PSUM banks, which idle until mid-kernel, so both tiles' forwards
pipeline; a few dummy matmuls at t=0 hold the PE HAM clock window busy
so the first transposes run at full clock.

K is never materialized: H0 = S @ (Wk @ w0) with the [D,D] composition
on-chip, and the retrieval's first layer is rewritten
  X1 = X0 @ w0 + (X0 @ Wk^T) @ M,   X0 = S @ wq
so X0 and P^T = Wk @ X0^T are computed early and only the tiny
M-eviction sits on the critical tail (U0/dW0 never materialize).
V is folded into H3's PSUM accumulation with a negated Wv (G3 raw =
H3 - V straight out of one bank; the 2/D scale lives in w3^T and in
the a3 bf16 cast).

All weights arrive in ONE packed DRAM tensor (HWDGE dispatch is ~625ns
per dma_start, serialized); seq arrives in 2 halves plus a casting
SWDGE bf16 copy.

ACT-table discipline: all forward Silus before any Derivative_silu
(H0..H2 evicted to SBUF), and a dummy Silu reloads the silu table
during the dW phase so the retrieval tail pays no table load.
"""

import numpy as np

import concourse.bass as bass
import concourse.mybir as mybir
import concourse.tile as tile
from concourse import bacc
from concourse.bass import ts
from concourse.bass_utils import run_bass_kernel_spmd
from concourse.masks import make_identity

B, N, D = 4, 1024, 128
DEPTH = 4
NCORES = 8
NT = 512            # tokens retrieved per core (half a batch)
TT = 512            # store-phase token tile
NTI = N // TT       # store tiles
NCHUNK = N // 128   # 8 token chunks of 128
RH = 256            # retrieval sub-tile
WPACK = 4 * D + D + 2 * D   # w0..w3 | wq | wkv

f32 = mybir.dt.float32
f32r = mybir.dt.float32r
bf16 = mybir.dt.bfloat16

AF = mybir.ActivationFunctionType
ALU = mybir.AluOpType

TM_DT = bf16


def _build_program(reps=1):
    nc = bacc.Bacc(
        "TRN2",
        target_bir_lowering=False,
        debug=False,
        enable_asserts=False,
        num_devices=NCORES,
    )

    seq = nc.dram_tensor("seq", [N, D], f32, kind="ExternalInput").ap()
    wp_dr = nc.dram_tensor("wpack", [D, WPACK], f32, kind="ExternalInput").ap()
    out_dr = nc.dram_tensor("out", [NT, D], f32, kind="ExternalOutput").ap()

    with tile.TileContext(nc) as tc:
        for _ in range(reps):
            _emit(tc, seq, wp_dr, out_dr)

    nc.compile()
    return nc


def _emit(tc, seq, wp_dr, out_dr):
    nc = tc.nc
    from contextlib import ExitStack

    from concourse.tile_rust import add_dep_helper as _dep  # type: ignore

    with ExitStack() as ctx:
        consts = ctx.enter_context(tc.tile_pool(name="consts", bufs=1))
        big = ctx.enter_context(tc.tile_pool(name="big", bufs=1))
        # PSUM banks: mm(2) + hold(2) + tr(3) + dw(1) = 8
        pp = ctx.enter_context(tc.tile_pool(name="pp", bufs=1, space="PSUM"))

        def pmm(name, w=512):
            return pp.tile([128, w], f32, tag="mm", bufs=2, name=name)

        def phold(name, w=512):
            return pp.tile([128, w], f32, tag="hold", bufs=2, name=name)

        def ptr(name):
            return pp.tile([128, 512], TM_DT, tag="tr", bufs=3, name=name)

        # tiny scratch silu pulls the first ACT table load off the
        # critical path (runs during the DMAs)
        scr = consts.tile([128, 1], f32, tag="scr")
        scr2 = consts.tile([128, 1], f32, tag="scr2")
        nc.gpsimd.memset(scr[:], 0.0)
        nc.scalar.activation(scr2[:], scr[:], AF.Silu)

        # PE warm-up: keep the HAM clock window busy before real work so
        # the S^T transposes and first matmuls run at full clock
        wupa = consts.tile([128, 128], f32r, tag="wupa")
        nc.gpsimd.memset(wupa[:].bitcast(f32), 0.0)
        wupp = pp.tile([128, 512], f32, tag="tr", bufs=3, name="wupp")
        for k in range(3):
            nc.tensor.matmul(
                wupp[:, 0:128], wupa[:], wupa[:],
                skip_group_check=True,
            )

        ident = consts.tile([128, 128], f32, tag="ident")
        make_identity(nc, ident)
        ident_b = consts.tile([128, 128], bf16, tag="ident_b")
        nc.gpsimd.tensor_copy(ident_b[:], ident[:])

        # ---- DMAs ordered by need ----
        wp = consts.tile([D, WPACK], f32, tag="wp")
        nc.sync.dma_start(wp[:], wp_dr)
        w_sb = [wp[:, ts(i, D)] for i in range(4)]
        wq_sb = wp[:, ts(4, D)]
        wkv_sb = wp[:, 5 * D : 7 * D]

        s_tm = big.tile([128, NCHUNK, 128], f32, tag="s_tm")
        seq_r = seq.rearrange("(c p) d -> p c d", p=128)
        nc.sync.dma_start(s_tm[:, 0:4], seq_r[:, 0:4])
        nc.sync.dma_start(s_tm[:, 4:8], seq_r[:, 4:8])
        s_tmb = big.tile([128, NCHUNK, 128], bf16, tag="s_tmb")

        # persistent SBUF activations (feature-major)
        st = big.tile([128, N], f32r, tag="st")
        a1 = big.tile([128, N], f32r, tag="a1")
        a2 = big.tile([128, N], f32r, tag="a2")
        a3 = big.tile([128, N], f32r, tag="a3")
        hsb = big.tile([128, 3, N], f32, tag="hsb")     # H0..H2 in SBUF
        sp0 = big.tile([128, N], f32, tag="sp0")
        sp1 = big.tile([128, N], f32, tag="sp1")
        sp2 = big.tile([128, N], f32, tag="sp2")
        g1 = big.tile([128, N], f32r, tag="g1")
        g2 = big.tile([128, N], f32r, tag="g2")
        g3 = big.tile([128, N], f32r, tag="g3")         # raw H3 - V
        # bf16 copies for the dW path (a3b carries the 2/D scale)
        a1b = big.tile([128, N], TM_DT, tag="a1b")
        a2b = big.tile([128, N], TM_DT, tag="a2b")
        a3b = big.tile([128, N], TM_DT, tag="a3b")
        g0b = big.tile([128, N], TM_DT, tag="g0b")
        g1b = big.tile([128, N], TM_DT, tag="g1b")
        g2b = big.tile([128, N], TM_DT, tag="g2b")
        g3b = big.tile([128, N], TM_DT, tag="g3b")

        wt = big.tile([128, 3, 128], f32r, tag="wt")    # w1^T,w2^T,w3^T*(2/D)
        wk_t = big.tile([128, 128], f32, tag="wk_t")    # Wk^T (fp32)
        wk_tr = big.tile([128, 128], f32r, tag="wk_tr")  # Wk^T (f32r)
        w0eff = big.tile([128, 128], f32r, tag="w0eff")  # Wk @ w0
        w0r = big.tile([128, 128], f32r, tag="w0r")
        wqr = big.tile([128, 128], f32r, tag="wqr")
        wv_r = big.tile([D, D], f32r, tag="wv_r")       # -Wv
        w_r = [None] + [
            big.tile([D, D], f32r, name=f"wr{i}", tag=f"wr{i}") for i in (1, 2, 3)
        ]
        for i in (1, 2, 3):
            nc.vector.tensor_copy(w_r[i][:], w_sb[i])
        # negated so V accumulates as -V into H3's PSUM bank
        nc.vector.tensor_scalar_mul(wv_r[:], wkv_sb[:, D : 2 * D], -1.0)
        nc.vector.tensor_copy(w0r[:], w_sb[0])
        nc.vector.tensor_copy(wqr[:], wq_sb)

        silu_insts = []
        dsilu_insts = []

        # ---- S^T (before weight-gated work: pool slots stay free) (fp32 PE transposes, evictions round to f32r) ----
        for g in range(NCHUNK // 4):
            p = pmm(f"p_st{g}")
            for j in range(4):
                nc.tensor.transpose(p[:, ts(j, 128)], s_tm[:, g * 4 + j], ident)
            nc.vector.tensor_copy(st[:, ts(g, 512)], p[:])

        # ---- setup transposes + W0eff ----
        p = pmm("p_tr1")
        nc.tensor.transpose(p[:, ts(0, 128)], wkv_sb[:, 0:D], ident)
        for i in range(2):
            nc.tensor.transpose(p[:, ts(1 + i, 128)], w_sb[1 + i], ident)
        nc.tensor.transpose(p[:, ts(3, 128)], w_sb[3], ident)
        nc.vector.tensor_copy(wk_t[:], p[:, 0:128])
        nc.vector.tensor_copy(wk_tr[:], p[:, 0:128])
        nc.vector.tensor_copy(
            wt[:, 0:2], p[:, 128:384].rearrange("p (c d) -> p c d", d=128)
        )
        nc.scalar.activation(wt[:, 2], p[:, 384:512], AF.Copy, scale=2.0 / D)

        p = pmm("p_w0eff")
        nc.tensor.matmul(p[:, 0:128], wk_t[:], w_sb[0])
        nc.vector.tensor_copy(w0eff[:], p[:, 0:128])

        # ---- X0^T = wq^T S^T and P^T = Wk X0^T (ACT evictions: DVE is the
        # fwd-setup bottleneck and ACT idles until the first Silu) ----
        x0 = big.tile([128, NT], f32r, tag="x0")
        px = pmm("p_x0")
        nc.tensor.matmul(px[:], wqr[:], st[:, 0:NT])
        nc.vector.tensor_copy(x0[:], px[:])
        pt = big.tile([128, NT], f32r, tag="pt")
        px = pmm("p_pt")
        nc.tensor.matmul(px[:], wk_tr[:], x0[:])
        nc.vector.tensor_copy(pt[:], px[:])

        # ---- forward: all Silus first; H2 held in PSUM, H0/H1 to SBUF ----
        holds = {}
        for t in range(NTI):
            sl = ts(t, TT)
            hloc = []
            for li in range(3):
                wst = (w0eff, w_r[1], w_r[2])[li]
                rhs = (st, a1, a2)[li]
                if li == 2:
                    h = phold(f"h{li}_{t}", TT)
                elif t == 1:
                    # tile-1 H0/H1 borrow the (idle until dW) tr banks
                    h = pp.tile([128, TT], f32, tag="tr", bufs=3, name=f"h{li}_{t}")
                else:
                    h = pmm(f"h{li}_{t}", TT)
                nc.tensor.matmul(h[:], wst[:], rhs[:, sl])
                dst = (a1, a2, a3)[li]
                silu_insts.append(nc.scalar.activation(dst[:, sl], h[:], AF.Silu))
                if li == 2:
                    hloc.append(h[:])
                else:
                    nc.vector.tensor_copy(hsb[:, li, sl], h[:])
                    hloc.append(hsb[:, li, sl])
            # H3 - V accumulated in one PSUM bank (wv_r is negated)
            if t == 1:
                h3 = pp.tile([128, TT], f32, tag="tr", bufs=3, name=f"h3_{t}")
            else:
                h3 = pmm(f"h3_{t}", TT)
            nc.tensor.matmul(h3[:], w_r[3][:], a3[:, sl], start=True, stop=False)
            nc.tensor.matmul(h3[:], wv_r[:], st[:, sl], start=False, stop=True)
            nc.vector.tensor_copy(g3[:, sl], h3[:])     # raw H3 - V
            nc.gpsimd.tensor_copy(a1b[:, sl], a1[:, sl].bitcast(f32))
            nc.gpsimd.tensor_copy(a2b[:, sl], a2[:, sl].bitcast(f32))
            nc.gpsimd.tensor_scalar_mul(
                a3b[:, sl], a3[:, sl].bitcast(f32), 2.0 / D
            )
            nc.gpsimd.tensor_copy(g3b[:, sl], g3[:, sl].bitcast(f32))
            holds[t] = hloc

        # ---- backward: Derivative_silu after all Silus + chains ----
        for t in range(NTI):
            di = nc.scalar.activation(
                sp2[:, ts(t, TT)], holds[t][2], AF.Derivative_silu
            )
            dsilu_insts.append(di)
        dsilu_insts.append(
            nc.scalar.activation(sp1[:], hsb[:, 1, :], AF.Derivative_silu)
        )
        dsilu_insts.append(
            nc.scalar.activation(sp0[:], hsb[:, 0, :], AF.Derivative_silu)
        )
        for t in range(NTI):
            sl = ts(t, TT)

            c2 = pmm(f"c2_{t}", TT)
            nc.tensor.matmul(c2[:], wt[:, 2], g3[:, sl])
            nc.vector.tensor_mul(g2[:, sl], c2[:], sp2[:, sl])

            c1 = pmm(f"c1_{t}", TT)
            nc.tensor.matmul(c1[:], wt[:, 1], g2[:, sl])
            nc.vector.tensor_mul(g1[:, sl], c1[:], sp1[:, sl])

            c0 = pmm(f"c0_{t}", TT)
            nc.tensor.matmul(c0[:], wt[:, 0], g1[:, sl])
            nc.vector.tensor_mul(g0b[:, sl], c0[:], sp0[:, sl])  # bf16 direct
            nc.gpsimd.tensor_copy(g2b[:, sl], g2[:, sl].bitcast(f32))
            nc.gpsimd.tensor_copy(g1b[:, sl], g1[:, sl].bitcast(f32))

        for di in dsilu_insts:
            _dep(di.ins, silu_insts[-1].ins, sync=False, reason="act-table order")

        # bf16 seq copy for the M matmuls -- held back (dep on the first
        # Silu) so its transfer doesn't delay the seq/weight DMAs at startup
        _stmb_dma = nc.gpsimd.dma_start(s_tmb[:], seq_r)
        _dep(_stmb_dma.ins, silu_insts[0].ins, sync=False,
             reason="defer bf16 seq copy off the startup DMA path")

        # ---- token-major transposes + dW accumulation ---------------------
        a_tm = [None] + [
            big.tile([128, N], TM_DT, name=f"atm{i}", tag=f"atm{i}") for i in (1, 2, 3)
        ]
        g_tm = [
            big.tile([128, N], TM_DT, name=f"gtm{i}", tag=f"gtm{i}") for i in range(4)
        ]
        u = [
            None,
            consts.tile([D, D], f32r, name="u1", tag="u1"),
            consts.tile([D, D], f32r, name="u2", tag="u2"),
            consts.tile([D, D], f32r, name="u3", tag="u3"),
        ]

        # reload the silu table during the dW phase, off the tail
        scr3 = consts.tile([128, 1], f32, tag="scr3")
        dummy = nc.scalar.activation(scr3[:], scr[:], AF.Silu)
        _dep(dummy.ins, dsilu_insts[-1].ins, sync=False, reason="act-table order")

        evict_flip = [0]

        def transpose_half(src, dst, h, name):
            p = ptr(name)
            for j in range(4):
                c = h * 4 + j
                nc.tensor.matmul(
                    p[:, ts(j, 128)], src[:, ts(c, 128)], ident_b[:],
                    is_transpose=True,
                )
            if evict_flip[0] % 3 == 2:
                nc.scalar.activation(dst[:, ts(h, 512)], p[:], AF.Copy)
            else:
                nc.vector.tensor_copy(dst[:, ts(h, 512)], p[:])
            evict_flip[0] += 1

        # dW3/dW2/dW1 and M share one PSUM bank (one accumulation group)
        acc = pp.tile([128, 4, 128], f32, tag="dw", bufs=1, name="dwacc")
        first = [True]

        for i, (ab, gb, atm, gtm, slot) in enumerate(
            (
                (a3b, g3b, a_tm[3], g_tm[3], 0),
                (a2b, g2b, a_tm[2], g_tm[2], 1),
                (a1b, g1b, a_tm[1], g_tm[1], 2),
            )
        ):
            for h in range(2):
                transpose_half(ab, atm, h, f"p_a{i}{h}")
                transpose_half(gb, gtm, h, f"p_g{i}{h}")
                for j in range(4):
                    c = h * 4 + j
                    nc.tensor.matmul(
                        acc[:, slot],
                        atm[:, ts(c, 128)],
                        gtm[:, ts(c, 128)],
                        start=first[0],
                        stop=False,
                    )
                    first[0] = False

        # M = S^T @ G0 into acc slot 3 (last writes of the bank group)
        for h in range(2):
            transpose_half(g0b, g_tm[0], h, f"p_g0{h}")
            for j in range(4):
                c = h * 4 + j
                nc.tensor.matmul(
                    acc[:, 3],
                    s_tmb[:, c],
                    g_tm[0][:, ts(c, 128)],
                    start=False,
                    stop=(h == 1 and j == 3),
                )
        m_r = big.tile([128, 128], f32r, tag="m_r")
        nc.vector.tensor_copy(m_r[:], acc[:, 3])
        for slot, i in ((2, 1), (1, 2), (0, 3)):
            nc.vector.tensor_add(u[i][:], acc[:, slot], w_sb[i])

        # ---- retrieval: X1 = X0 @ w0 + P @ M, then layers 2..4 ------------
        r1 = big.tile([128, NT], f32r, tag="r1")
        r2 = big.tile([128, NT], f32r, tag="r2")
        r3 = big.tile([128, NT], f32r, tag="r3")
        o_tm = big.tile([128, NT // 128, 128], f32, tag="o_tm")
        out_r = out_dr.rearrange("(c p) d -> p c d", p=128)

        nh = NT // RH
        px1s = []
        for hh in range(nh):
            sl = ts(hh, RH)
            px = phold(f"px1_{hh}", RH)
            # term 1 (X0 @ w0) has no M dependency -- runs during the dW phase
            nc.tensor.matmul(px[:], w0r[:], x0[:, sl], start=True, stop=False)
            px1s.append(px)
        for hh in range(nh):
            sl = ts(hh, RH)
            px = px1s[hh]
            nc.tensor.matmul(px[:], m_r[:], pt[:, sl], start=False, stop=True)
            nc.scalar.activation(r1[:, sl], px[:], AF.Silu)
        for hh in range(nh):
            sl = ts(hh, RH)
            px = phold(f"px2_{hh}", RH)
            nc.tensor.matmul(px[:], u[1][:], r1[:, sl])
            nc.scalar.activation(r2[:, sl], px[:], AF.Silu)
        for hh in range(nh):
            sl = ts(hh, RH)
            px = pmm(f"px3_{hh}", RH)
            nc.tensor.matmul(px[:], u[2][:], r2[:, sl])
            nc.scalar.activation(r3[:, sl], px[:], AF.Silu)
        for hh in range(nh):
            po = pmm(f"po{hh}", RH)
            for j in range(RH // 128):
                c = hh * (RH // 128) + j
                nc.tensor.matmul(
                    po[:, ts(j, 128)],
                    r3[:, ts(c, 128)],
                    u[3][:],
                    start=(j == 0),
                    stop=(j == RH // 128 - 1),
                )
            nc.vector.tensor_copy(
                o_tm[:, 2 * hh : 2 * hh + 2],
                po[:].rearrange("p (c d) -> p c d", d=128),
            )
            nc.sync.dma_start(
                out_r[:, 2 * hh : 2 * hh + 2], o_tm[:, 2 * hh : 2 * hh + 2]
            )


_CACHE = {}


def _get_nc():
    if "nc" not in _CACHE:
        _CACHE["nc"] = _build_program()
    return _CACHE["nc"]


def kernel(seq, w0, w1, w2, w3, wq, wkv):
    nc = _get_nc()
    seq = np.ascontiguousarray(np.asarray(seq, np.float32))
    wpack = np.ascontiguousarray(
        np.concatenate(
            [np.asarray(x, np.float32) for x in (w0, w1, w2, w3, wq, wkv)], axis=1
        )
    )

    in_maps = []
    for c in range(NCORES):
        b, h = c // 2, c % 2
        if h == 0:
            s = seq[b]
        else:
            # rotate: retrieval half first; grad sum is order-invariant
            s = np.concatenate([seq[b, NT:], seq[b, :NT]], axis=0)
        in_maps.append({"seq": np.ascontiguousarray(s), "wpack": wpack})

    res = run_bass_kernel_spmd(nc, in_maps, core_ids=list(range(NCORES)))
    _CACHE["last_results"] = res

    out = np.empty((B, N, D), np.float32)
    for c in range(NCORES):
        b, h = c // 2, c % 2
        out[b, h * NT : (h + 1) * NT] = res.results[c]["out"]
    return out

